# revision 1
# baseline (speedup 1.0000x reference)
"""Self-contained TRN2 Bass kernel for nn_Attention (B=4, N=2048, D=1024, H=16).

Sharding: 8 NeuronCores, core c = (batch b = c//2, head-half = c%2).
Each core computes causal attention for its batch and 8 of 16 heads plus the
row-parallel half of the output projection; the host sums the two half-partials
per batch.

Per-core pipeline (all on-device, Tile-scheduled):
  xT [D, NT] f32  --qk-proj (fp32r matmuls)-->  qT/kT [128 feats, NT] bf16
                  --v-proj--->                  V [token, feat] bf16 (+ones col)
  S^T chunk [128 k-tok, 512 q-tok] = kT-part @ qT   (bf16, f32 psum)
  exp on ScalarE; causal triangle mask added on diagonal blocks (DVE)
  O^T + softmax denominator via P^T @ [V | 1]
  normalize by 1/den (DVE recip + DRAM-bounce partition broadcast)
  partial out [NT, E] = OT.T @ woutT  (bf16 matmuls, f32 psum)
"""

import os
import sys
import types
from contextlib import ExitStack
from dataclasses import dataclass

for _p in ('/opt/trn_rl_repo', '/root/.axon_site/_ro/trn_rl_repo'):
    if os.path.isdir(_p) and _p not in sys.path:
        sys.path.append(_p)

import numpy as np
import ml_dtypes

import concourse.bass as bass
import concourse.mybir as mybir
import concourse.tile as tile
from concourse import bacc

F32 = mybir.dt.float32
F32R = mybir.dt.float32r
BF16 = mybir.dt.bfloat16


# ---------------------------------------------------------------- harness fixes
def _install_ntff_hook():
    """Register the axon NTFF profile hook that trn_boot skips when the
    container's antenv stub lacks axon_hooks (needed only for trace=True)."""
    if 'antenv.axon_hooks' in sys.modules:
        return
    try:
        import antenv
        mod = types.ModuleType('antenv.axon_hooks')
        _hook = [None]
        mod.set_axon_ntff_profile_hook = lambda h: _hook.__setitem__(0, h)
        mod.get_axon_ntff_profile_hook = lambda: _hook[0]
        sys.modules['antenv.axon_hooks'] = mod
        antenv.axon_hooks = mod
        from trn_agent_boot.trn_boot import _ntff_profile_via_ctypes
        so = '/opt/axon/libaxon_pjrt.so'
        if os.path.exists(so):
            hook = _ntff_profile_via_ctypes(so)
            if hook is not None:
                mod.set_axon_ntff_profile_hook(hook)
    except Exception:
        pass


def _patch_tile_drain():
    """walrus TPB_CTRL encodes <=2 sync waits; Tile's tail drain can carry
    more. Split extras onto single-wait nops (sequentially equivalent)."""
    import concourse.tile as tile_mod
    if getattr(tile_mod.TileContext, '_drain_patched', False):
        return
    from concourse.tile import ScopedClock

    def _drain_and_barrier(self, tick_clock, wait_clock):
        nc = self.nc
        drain_inst = nc.sync.drain()
        wait_clock.add_sem_waits(
            drain_inst.ins, ScopedClock({None: tick_clock.global_clock}))
        si = drain_inst.ins.sync_info
        if si is not None and si.on_wait and len(si.on_wait) > 1:
            waits = list(si.on_wait)
            drain_inst.ins.sync_info = mybir.SyncInfo(
                on_wait=waits[:1], on_update=list(si.on_update or []))
            for w in waits[1:]:
                nop = nc.sync.nop(nofuse=True)
                nop.ins.sync_info = mybir.SyncInfo(on_wait=[w], on_update=[])
        nc.all_engine_barrier()
        popped = nc._tile_sem_poison_stack.pop()
        assert popped is self._sem_poison
        nc.clear_and_free_semaphores(list(self.sems.allocated().values()))
        nc.all_engine_barrier()

    tile_mod.TileContext._drain_and_barrier = _drain_and_barrier
    tile_mod.TileContext._drain_patched = True


# ---------------------------------------------------------------- kernel build
@dataclass(frozen=True)
class Cfg:
    NT: int = 2048   # tokens
    D: int = 1024    # model dim
    HH: int = 8      # heads per core
    DH: int = 64     # head dim
    E: int = 1024    # output features
    QC: int = 512    # q-chunk (free dim of score tiles)
    KC: int = 128    # k-tile (partition dim of score tiles)
    MASK_NEG: float = -1e30
    PT_BUFS: int = 8
    PSS_BUFS: int = 2
    PSO_BUFS: int = 2
    OT_BUFS: int = 8
    DEN_BUFS: int = 6
    XT_BUFS: int = 2
    PSA_BUFS: int = 4

    @property
    def DC(self): return self.D // 128
    @property
    def NKT(self): return self.NT // self.KC
    @property
    def NQC(self): return self.NT // self.QC
    @property
    def NPAIR(self): return self.HH // 2
    @property
    def QF(self): return self.HH * self.DH
    @property
    def VW(self): return self.DH + 1


def build(cfg: Cfg) -> bass.Bass:
    _patch_tile_drain()
    nc = bacc.Bacc('TRN2', target_bir_lowering=False)
    c = cfg
    assert c.QC % c.KC == 0 and c.NT % c.QC == 0 and c.D % 128 == 0
    assert c.DH * 2 == c.KC
    JPT = c.QC // c.KC

    xT = nc.declare_dram_parameter("xT", [c.D, c.NT], BF16, isOutput=False)
    wqk = nc.declare_dram_parameter("wqk", [c.DC, 128, 2 * c.QF], BF16, isOutput=False)
    wv = nc.declare_dram_parameter("wv", [c.DC, 128, c.QF], BF16, isOutput=False)
    wout = nc.declare_dram_parameter("wout", [128, c.NPAIR, c.E], BF16, isOutput=False)
    mtri = nc.declare_dram_parameter("mtri", [128, c.KC], BF16, isOutput=False)
    iden = nc.declare_dram_parameter("iden", [128, 128], BF16, isOutput=False)
    out = nc.declare_dram_parameter("out", [c.NT, c.E], F32, isOutput=True)

    with tile.TileContext(nc) as tc, ExitStack() as ctx:
        const = ctx.enter_context(tc.tile_pool(name="const", bufs=1))
        persist = ctx.enter_context(tc.tile_pool(name="persist", bufs=1))

        ones64 = const.tile([c.VW, c.DH], F32)
        nc.vector.memset(ones64[c.DH:c.VW, :], 1.0)
        mtri_sb = const.tile([128, c.KC], BF16)
        nc.sync.dma_start(out=mtri_sb[:], in_=mtri[:])
        iden_sb = const.tile([128, 128], BF16)
        nc.sync.dma_start(out=iden_sb[:], in_=iden[:])
        wout_sb = const.tile([128, c.NPAIR, c.E], BF16)
        nc.sync.dma_start(out=wout_sb[:], in_=wout[:])
        wv_sb = const.tile([128, c.DC, c.QF], BF16)
        for dc in range(c.DC):
            nc.sync.dma_start(out=wv_sb[:, dc, :], in_=wv[dc])
        wqk_sb = const.tile([128, c.DC, 2 * c.QF], BF16)
        for dc in range(c.DC):
            nc.sync.dma_start(out=wqk_sb[:, dc, :], in_=wqk[dc])

        qk_sb = [persist.tile([128, c.NT], BF16, tag=f"qk{e}", name=f"qk{e}")
                 for e in range(2 * c.NPAIR)]
        V_sb = persist.tile([128, c.NKT, c.HH, c.VW], BF16, tag="V", name="V_sb")
        nc.vector.memset(V_sb[:, :, :, c.DH], 1.0)
        OT_sb = [persist.tile([128, c.NT], BF16, tag=f"ot{p}", name=f"ot{p}")
                 for p in range(c.NPAIR)]

        xt_t = [persist.tile([128, c.NT], BF16, tag=f"xt{dc}", name=f"xt{dc}")
                for dc in range(c.DC)]
        for half in range(2):
            hs = slice(half * (c.NT // 2), (half + 1) * (c.NT // 2))
            for dc in range(c.DC):
                nc.sync.dma_start(
                    out=xt_t[dc][:, hs], in_=xT[dc * 128:(dc + 1) * 128, hs])

        # ---------------- Phase A: v-projection (first q-chunk only;
        # the rest is emitted as filler work inside pair 0) ----------------
        NTT0 = c.QC // 128
        with tc.tile_pool(name="ps_a", bufs=c.PSA_BUFS, space="PSUM") as ps_a:
            for nt in range(min(NTT0, c.NKT)):
                psv = ps_a.tile([128, c.QF], F32, tag="ps", name="psv")
                for dc in range(c.DC):
                    nc.tensor.matmul(
                        psv[:],
                        lhsT=xt_t[dc][:, nt * 128:(nt + 1) * 128],
                        rhs=wv_sb[:, dc, :],
                        start=(dc == 0), stop=(dc == c.DC - 1),
                    )
                nc.vector.tensor_copy(
                    out=V_sb[:, nt, :, 0:c.DH],
                    in_=psv[:].rearrange("p (h f) -> p h f", h=c.HH),
                )

        # ---------------- Phase B: attention ----------------
        with (
            tc.tile_pool(name="pt", bufs=c.PT_BUFS) as pt_pool,
            tc.tile_pool(name="otst", bufs=c.OT_BUFS) as ot_pool,
            tc.tile_pool(name="den", bufs=c.DEN_BUFS) as den_pool,
            tc.tile_pool(name="dend", bufs=4, space="DRAM") as dend_pool,
            tc.tile_pool(name="ps_s", bufs=c.PSS_BUFS, space="PSUM") as ps_s,
            tc.tile_pool(name="ps_o", bufs=c.PSO_BUFS, space="PSUM") as ps_o,
            tc.tile_pool(name="ps_f", bufs=2, space="PSUM") as ps_f,
            tc.tile_pool(name="osbB", bufs=3) as outB_pool,
        ):
            def evac_stage1(psO):
                ocps = []
                for h2 in range(2):
                    # evacuate psum to SBUF immediately to free the bank
                    ocp = ot_pool.tile([c.VW, c.QC], F32, tag="ocp",
                                       name="ocp", bufs=6)
                    nc.vector.tensor_copy(out=ocp[:], in_=psO[h2][:])
                    ocps.append(ocp)
                return ocps

            def evac_stage2(p, t, ocps):
                qsl_full = slice(t * c.QC, (t + 1) * c.QC)
                for h2 in range(2):
                    ocp = ocps[h2]
                    den_t = den_pool.tile([c.VW, c.QC], F32, tag="den",
                                          name="den_t")
                    # 1/den = exp(-ln(den)) on ScalarE
                    nc.scalar.activation(
                        out=den_t[c.DH:c.VW, :], in_=ocp[c.DH:c.VW, :],
                        func=mybir.ActivationFunctionType.Ln)
                    nc.scalar.activation(
                        out=den_t[c.DH:c.VW, :], in_=den_t[c.DH:c.VW, :],
                        func=mybir.ActivationFunctionType.Exp, scale=-1.0)
                    # partition broadcast via DRAM bounce
                    den_d = dend_pool.tile([c.QC], F32, tag="dend",
                                           name="den_d")
                    nc.sync.dma_start(out=den_d[:], in_=den_t[c.DH:c.VW, :])
                    divB = den_pool.tile([c.DH, c.QC], F32, tag="div",
                                         name="divB")
                    nc.sync.dma_start(
                        out=divB[:], in_=den_d.partition_broadcast(c.DH))
                    if h2 == 0:
                        nc.vector.tensor_tensor(
                            out=OT_sb[p][0:c.DH, qsl_full], in0=ocp[0:c.DH, :],
                            in1=divB[:], op=mybir.AluOpType.mult,
                        )
                    else:
                        ot_st = ot_pool.tile([c.DH, c.QC], BF16, tag="ot",
                                             name="ot_st")
                        nc.vector.tensor_tensor(
                            out=ot_st[:], in0=ocp[0:c.DH, :], in1=divB[:],
                            op=mybir.AluOpType.mult,
                        )
                        nc.sync.dma_start(
                            out=OT_sb[p][c.DH:2 * c.DH, qsl_full],
                            in_=ot_st[:])

            def emit_proj_chunk(nt, ec):
                esl = slice(ec * c.QC, (ec + 1) * c.QC)
                psP = ps_f.tile([128, c.QC], F32, tag="f", name="psP")
                for pr in range(c.NPAIR):
                    nc.tensor.matmul(
                        psP[:],
                        lhsT=OT_sb[pr][:, nt * 128:(nt + 1) * 128],
                        rhs=wout_sb[:, pr, esl],
                        start=(pr == 0), stop=(pr == c.NPAIR - 1),
                    )
                o_sb = outB_pool.tile([128, c.QC], F32, tag="ob", name="o_sb")
                nc.vector.tensor_copy(out=o_sb[:], in_=psP[:])
                nc.sync.dma_start(
                    out=out[nt * 128:(nt + 1) * 128, esl], in_=o_sb[:])

            def emit_qk_chunk(pp, ci):
                e = (pp, c.NPAIR + pp)[ci // c.NQC]
                ncc = ci % c.NQC
                nsl = slice(ncc * c.QC, (ncc + 1) * c.QC)
                psqk = ps_f.tile([128, c.QC], F32, tag="f", name="psqk")
                for dc in range(c.DC):
                    nc.tensor.matmul(
                        psqk[:],
                        lhsT=wqk_sb[:, dc, e * 128:(e + 1) * 128],
                        rhs=xt_t[dc][:, nsl],
                        start=(dc == 0), stop=(dc == c.DC - 1),
                    )
                nc.vector.tensor_copy(out=qk_sb[e][:, nsl], in_=psqk[:])

            def emit_v_chunk(nt):
                psv = ps_f.tile([128, c.QF], F32, tag="f", name="psvf")
                for dc in range(c.DC):
                    nc.tensor.matmul(
                        psv[:],
                        lhsT=xt_t[dc][:, nt * 128:(nt + 1) * 128],
                        rhs=wv_sb[:, dc, :],
                        start=(dc == 0), stop=(dc == c.DC - 1),
                    )
                nc.vector.tensor_copy(
                    out=V_sb[:, nt, :, 0:c.DH],
                    in_=psv[:].rearrange("p (h f) -> p h f", h=c.HH),
                )

            pending2 = None
            # prologue: only the first q-chunk's columns of q and k
            emit_qk_chunk(0, 0)
            emit_qk_chunk(0, c.NQC)
            for p in range(c.NPAIR):
                q_t, k_t = qk_sb[p], qk_sb[c.NPAIR + p]
                for t in range(c.NQC):
                    njt = JPT * t + JPT
                    psO = [ps_o.tile([c.VW, c.QC], F32, tag="o", name=f"psO{_h}")
                           for _h in range(2)]
                    def emit_pv(items):
                        for (h2_, pt_, lo_, j_) in items:
                            nc.tensor.matmul(
                                psO[h2_][:, lo_:],
                                lhsT=V_sb[:, j_, 2 * p + h2_, :],
                                rhs=pt_[:, h2_, lo_:],
                                start=(j_ == 0), stop=(j_ == njt - 1),
                            )

                    pipe = []
                    for j in range(njt):
                        off = j * c.KC - t * c.QC
                        band = off >= 0
                        lo = max(off, 0)
                        jsl = slice(j * c.KC, (j + 1) * c.KC)
                        qsl = slice(t * c.QC + lo, (t + 1) * c.QC)
                        # both heads' scores into one 2-bank psum tile
                        psS = ps_s.tile([128, 2, c.QC], F32, tag="s", name="psS")
                        for h2 in range(2):
                            hsl = slice(h2 * c.DH, (h2 + 1) * c.DH)
                            nc.tensor.matmul(
                                psS[:, h2, lo:], lhsT=k_t[hsl, jsl],
                                rhs=q_t[hsl, qsl], start=True, stop=(not band),
                            )
                            if band:
                                # causal mask add on PE: psum += I.T @ mtri
                                nc.tensor.matmul(
                                    psS[:, h2, off:off + c.KC], lhsT=iden_sb[:],
                                    rhs=mtri_sb[:], start=False, stop=True,
                                )
                        pt_t = pt_pool.tile([128, 2, c.QC], BF16, tag="pt",
                                            name="pt_t")
                        nc.scalar.activation(
                            out=pt_t[:, 0, lo:].rearrange(
                                "p q -> p q") if False else pt_t[:, :, lo:],
                            in_=psS[:, :, lo:],
                            func=mybir.ActivationFunctionType.Exp,
                        )
                        pipe.append([(0, pt_t, lo, j), (1, pt_t, lo, j)])
                        if len(pipe) > 3:
                            emit_pv(pipe.pop(0))
                    # interleave filler work (next pair's qk projection,
                    # or output-projection chunks during the last pair) with
                    # the PV pipe flush so the PE has matmuls to run while
                    # ScalarE finishes the tail exps
                    NTT = c.QC // 128
                    if p == 0:
                        fillers = []
                        if t + 1 < c.NQC:
                            # next q-chunk's V rows and q/k columns
                            fillers += [
                                lambda nt_=nt_: emit_v_chunk(nt_)
                                for nt_ in range((t + 1) * NTT,
                                                 min((t + 2) * NTT, c.NKT))]
                            fillers += [
                                lambda ci=ci: emit_qk_chunk(0, ci)
                                for ci in (t + 1, c.NQC + t + 1)]
                        fillers += [lambda i=i: emit_qk_chunk(1, 2 * t + i)
                                    for i in range(2)]
                    elif p + 1 < c.NPAIR:
                        fillers = [lambda i=i: emit_qk_chunk(p + 1, 2 * t + i)
                                   for i in range(2)]
                    elif t >= 2:
                        tp = t - 2
                        fillers = [
                            lambda nt_=nt_, ec_=ec_: emit_proj_chunk(nt_, ec_)
                            for nt_ in range(tp * NTT, (tp + 1) * NTT)
                            for ec_ in range(c.E // c.QC)]
                    else:
                        fillers = []
                    while fillers or pipe:
                        if fillers:
                            fillers.pop(0)()
                        if pipe:
                            emit_pv(pipe.pop(0))
                    ocps = evac_stage1(psO)
                    if pending2 is not None:
                        evac_stage2(*pending2)
                    pending2 = (p, t, ocps)

            if pending2 is not None:
                evac_stage2(*pending2)

        # ---------------- Phase C: output projection ----------------
        with (
            tc.tile_pool(name="osb", bufs=3) as out_pool,
            tc.tile_pool(name="ps_c", bufs=4, space="PSUM") as ps_c,
        ):
            for nt in range(max(0, c.NQC - 2) * (c.QC // 128), c.NT // 128):
                for ec in range(c.E // c.QC):
                    esl = slice(ec * c.QC, (ec + 1) * c.QC)
                    psP = ps_c.tile([128, c.QC], F32, tag="pp", name="psP")
                    for p in range(c.NPAIR):
                        nc.tensor.matmul(
                            psP[:],
                            lhsT=OT_sb[p][:, nt * 128:(nt + 1) * 128],
                            rhs=wout_sb[:, p, esl],
                            start=(p == 0), stop=(p == c.NPAIR - 1),
                        )
                    o_sb = out_pool.tile([128, c.QC], F32, tag="ob", name="o_sb")
                    nc.vector.tensor_copy(out=o_sb[:], in_=psP[:])
                    nc.sync.dma_start(
                        out=out[nt * 128:(nt + 1) * 128, esl], in_=o_sb[:])

    nc.compile()
    return nc


# ---------------------------------------------------------------- host side
def make_core_inputs(xb, w_qkv, w_out, mask, cfg, half):
    c = cfg
    D = c.D
    scale = 1.0 / np.sqrt(c.DH)
    heads = range(half * c.HH, (half + 1) * c.HH)
    q_rows = np.concatenate(
        [w_qkv[h * c.DH:(h + 1) * c.DH, :] for h in heads]) * scale
    k_rows = np.concatenate(
        [w_qkv[D + h * c.DH:D + (h + 1) * c.DH, :] for h in heads])
    v_rows = np.concatenate(
        [w_qkv[2 * D + h * c.DH:2 * D + (h + 1) * c.DH, :] for h in heads])
    wqk_t = np.concatenate([q_rows, k_rows], axis=0).T  # [D, 2QF]
    wqk_t = np.ascontiguousarray(
        wqk_t.reshape(c.DC, 128, 2 * c.QF)).astype(ml_dtypes.bfloat16)
    wv_t = np.ascontiguousarray(
        v_rows.T.reshape(c.DC, 128, c.QF)).astype(ml_dtypes.bfloat16)
    wo = w_out[:, half * c.QF:(half + 1) * c.QF].T  # [QF, E]
    wo = np.ascontiguousarray(
        wo.reshape(c.NPAIR, 128, c.E).transpose(1, 0, 2)).astype(
            ml_dtypes.bfloat16)
    mt = np.where(mask[0, 0, :c.KC, :c.KC].T != 0, 0.0,
                  c.MASK_NEG).astype(ml_dtypes.bfloat16)
    return {
        "xT": np.ascontiguousarray(xb.T).astype(ml_dtypes.bfloat16),
        "wqk": wqk_t,
        "wv": wv_t,
        "wout": wo,
        "mtri": mt,
        "iden": np.eye(128, dtype=ml_dtypes.bfloat16),
    }


_CACHE = {}


def run_sharded(x, mask, w_qkv, w_out, trace=False, trace_cores=None):
    """Shard inputs over 8 cores, run the bass kernel, gather full output.
    Returns (out [B,N,D] f32, BassKernelResults)."""
    # the axon PJRT backend is required for execution; guard against a
    # caller environment that overrode JAX_PLATFORMS before jax init
    if 'jax' not in sys.modules and 'axon' not in os.environ.get(
            'JAX_PLATFORMS', 'axon'):
        os.environ['JAX_PLATFORMS'] = 'axon'
    from concourse.bass_utils import run_bass_kernel_spmd

    cfg = Cfg()
    B = x.shape[0]
    n_cores = 2 * B
    if 'nc' not in _CACHE:
        _CACHE['nc'] = build(cfg)
    nc = _CACHE['nc']

    x = np.asarray(x, np.float32)
    mask = np.asarray(mask)
    w_qkv = np.asarray(w_qkv, np.float32)
    w_out = np.asarray(w_out, np.float32)

    in_maps = []
    for core in range(n_cores):
        b, half = core // 2, core % 2
        in_maps.append(make_core_inputs(x[b], w_qkv, w_out, mask, cfg, half))

    if trace:
        _install_ntff_hook()
    res = run_bass_kernel_spmd(
        nc, in_maps, core_ids=list(range(n_cores)), trace=trace,
        trace_cores=trace_cores)
    outs = []
    for b in range(B):
        outs.append(res.results[2 * b]["out"].astype(np.float64)
                    + res.results[2 * b + 1]["out"].astype(np.float64))
    return np.stack(outs).astype(np.float32), res


def kernel(x, mask, w_qkv, w_out):
    out, _ = run_sharded(x, mask, w_qkv, w_out, trace=False)
    return out

